# revision 30
# baseline (speedup 1.0000x reference)
"""Trainium2 Bass kernel for nn_AttentionBlock (B=2, L=2048, D=1024, H=16).

Sharding: tensor-parallel over heads. Each of 8 cores computes 2 heads:
Wq/Wk/Wv column-sharded, Wo row-sharded; host sums the 8 partial outputs.

v3: all-x upfront multi-queue prefetch, bf16 ex/v PV datapath, full-PSUM
early drain copy (kills chunk-boundary WAR stalls), tapered final chunks
(512,512,512,384,128 on b1) to shrink the exposed tail drain.

v2 structure (per core, bf16 datapath, fp32 PSUM):
  - v stored [token, head*64+d] per 128-token block (PE transpose, no ones col)
  - scores: 2 heads row-packed (K=64) in one PE slot
  - PV: 2 heads col-packed (M=64 at cols 0/64) in one PE slot
  - softmax denominators: ones[128,64] lhsT broadcast matmuls, col-packed,
    accumulating [128,LC] PSUM that directly IS the per-head-row denominator
    layout -> reciprocal_approx_fast -> normalize fused into the PSUM drain
  - projections interleaved into attention slots (work queue) so the PE and
    ACT engines stay dense; exp table preloaded via a dummy activation
"""
import numpy as np
from contextlib import ExitStack
from collections import deque

import concourse.bacc as bacc
import concourse.tile as tile
import concourse.mybir as mybir
from concourse import bass_utils
from concourse.masks import make_identity

F32 = mybir.dt.float32
F32R = mybir.dt.float32r
BF16 = mybir.dt.bfloat16
AF = mybir.ActivationFunctionType
ALU = mybir.AluOpType

B, L, D, H, DH = 2, 2048, 1024, 16, 64
NCORES = 8
HPC = H // NCORES       # heads per core = 2
DHC = HPC * DH          # 128
KT = D // 128           # 8 k-tiles over the contraction dim


def build(Lb=L, debug=False):
    BLb = B * Lb
    NJT = Lb // 128            # key tiles per batch = 16
    LC = min(512, Lb)          # query-chunk width
    NLC = Lb // LC             # query chunks per batch = 4
    PC = min(512, BLb)         # projection chunk width
    NPC = BLb // PC            # projection chunks (global) = 8
    NTT = BLb // 128           # 128-token tiles (global) = 32

    nc = bacc.Bacc("TRN2", target_bir_lowering=False, debug=debug, num_devices=8)

    # host supplies SBUF-layout (partition-major, chunk-contiguous) arrays so
    # every DMA is contiguous per partition (fast descriptor gen + transfer)
    NPC_ = BLb // min(512, BLb)
    xT = nc.dram_tensor("xT", [128, NPC_, KT, min(512, BLb)], BF16,
                        kind="ExternalInput")
    wq = nc.dram_tensor("wq", [128, KT, DHC], BF16, kind="ExternalInput")
    wk = nc.dram_tensor("wk", [128, KT, DHC], BF16, kind="ExternalInput")
    wv = nc.dram_tensor("wv", [128, KT, DHC], BF16, kind="ExternalInput")
    wo = nc.dram_tensor("wo", [DHC, D], BF16, kind="ExternalInput")
    bq = nc.dram_tensor("bq", [DHC, 1], F32, kind="ExternalInput")
    bk = nc.dram_tensor("bk", [DHC, 1], F32, kind="ExternalInput")
    bv = nc.dram_tensor("bv", [DHC, 1], F32, kind="ExternalInput")
    out = nc.dram_tensor("out", [BLb, D], BF16, kind="ExternalOutput")

    xT_v = xT.ap()                                          # [128, NPC, KT, PC]
    wq_v = wq.ap()
    wk_v = wk.ap()
    wv_v = wv.ap()

    with tile.TileContext(nc) as tc, ExitStack() as ctx:
        # --- pools ---
        persist = ctx.enter_context(tc.tile_pool(name="persist", bufs=1))
        xpool = ctx.enter_context(tc.tile_pool(name="xchunk", bufs=NPC))
        vstage = ctx.enter_context(tc.tile_pool(name="vstage", bufs=2))
        expool = ctx.enter_context(tc.tile_pool(name="expool", bufs=5))
        drpool = ctx.enter_context(tc.tile_pool(name="drpool", bufs=2))
        outpool = ctx.enter_context(tc.tile_pool(name="outpool", bufs=3))
        # PSUM: sc 2banks x2 + aT 1 + den 1 + single 1x2 = 8 banks
        scpool = ctx.enter_context(tc.tile_pool(name="scpool", bufs=2, space="PSUM"))
        accpool = ctx.enter_context(tc.tile_pool(name="accpool", bufs=1, space="PSUM"))
        psing = ctx.enter_context(tc.tile_pool(name="psing", bufs=2, space="PSUM"))

        # --- persistent tiles ---
        qT_sb = persist.tile([128, BLb], BF16, tag="qT")
        kT_sb = persist.tile([128, BLb], BF16, tag="kT")
        v_sb = persist.tile([128, NTT, 2, 65], BF16, tag="v")
        aT_sb = [
            persist.tile([128, Lb], BF16, tag=f"aT{b}", name=f"aT{b}")
            for b in range(B)
        ]
        wq_sb = persist.tile([128, KT, DHC], BF16, tag="wq")
        wk_sb = persist.tile([128, KT, DHC], BF16, tag="wk")
        wv_sb = persist.tile([128, KT, DHC], BF16, tag="wv")
        wo_sb = persist.tile([DHC, D], BF16, tag="wo")
        bq_sb = persist.tile([DHC, 1], F32, tag="bq")
        bk_sb = persist.tile([DHC, 1], F32, tag="bk")
        bv_sb = persist.tile([DHC, 1], F32, tag="bv")
        ident = persist.tile([128, 128], BF16, tag="ident")
        ones64 = persist.tile([65, DH], BF16, tag="ones64")   # row 64 used
        dumm = persist.tile([1, 2], F32, tag="dumm")

        xt_tiles = {}
        vt_tiles = {}

        # --- prologue: weight loads, constants, ACT exp-table preload ---
        # wk/x0 split in halves so the first k-col matmuls start as soon as
        # the first half lands (~2.5us) instead of after the full loads
        for i in range(4):
            w4 = KT // 4
            nc.sync.dma_start(wk_sb[:, i * w4:(i + 1) * w4, :],
                              wk_v[:, i * w4:(i + 1) * w4, :])
        nc.sync.dma_start(wv_sb[:], wv_v)
        nc.sync.dma_start(wq_sb[:], wq_v)
        make_identity(nc, ident[:])
        nc.vector.memset(ones64[:], 1.0)
        nc.vector.memset(dumm[:], 0.0)
        nc.scalar.activation(dumm[:], dumm[:], AF.Exp)  # load exp table early
        # ones-augment columns of v (denominator trick)
        nc.vector.memset(v_sb[:, :, :, 64:65], 1.0)
        # PE warm-up: dummy matmuls while the first x/wk DMAs land. Keeps
        # the PE continuously busy so the 0.65->1.2->2.4GHz p-state ramp
        # completes before real work arrives.
        warm = psing.tile([128, 128], F32, tag="single", name="warm",
                          padded_shape=[128, 512])
        for i in range(16):
            nc.tensor.matmul(warm[:, :], ident[:], ident[:],
                             start=(i == 0), stop=(i == 15))

        # --- projection work items ---
        def x_load(chn, split=1):
            def f():
                xt = xpool.tile([128, KT, PC], BF16, tag="xt")
                # scalar-engine HWDGE queue: don't serialize behind the
                # sync-engine weight/output DMA issue stream. split=n
                # pieces let the first kt slices land sooner.
                w = KT // split
                for i in range(split):
                    nc.scalar.dma_start(
                        xt[:, i * w:(i + 1) * w, :],
                        xT_v[:, chn, i * w:(i + 1) * w, :],
                    )
                xt_tiles[chn] = xt
            return f

        ps_tiles = {}

        def proj_col(chn, which, half=None):
            # half=None: full 8-kt column; half=0/1: split into two 4-kt
            # bursts (smaller PE bursts keep ACT fed between slots)
            def f():
                xt = xt_tiles[chn]
                w_sb, b_sb = {
                    "q": (wq_sb, bq_sb), "k": (wk_sb, bk_sb), "v": (wv_sb, bv_sb)
                }[which]
                if half in (None, 0):
                    ps_tiles[(chn, which)] = psing.tile(
                        [128, PC], F32, tag="single", name=f"ps_{chn}_{which}"
                    )
                ps = ps_tiles[(chn, which)]
                kts = range(KT) if half is None else \
                    range(half * KT // 2, (half + 1) * KT // 2)
                for kt in kts:
                    nc.tensor.matmul(
                        ps[:, :], w_sb[:, kt, :], xt[:, kt, :],
                        start=(kt == 0), stop=(kt == KT - 1),
                    )
                if half == 0:
                    return
                if which == "q":
                    nc.vector.tensor_scalar(
                        qT_sb[:, chn * PC:(chn + 1) * PC], ps[:, :],
                        b_sb[:, 0:1], None, ALU.add,
                    )
                elif which == "k":
                    # split: scores of the first key-tiles unblock ~500ns
                    # sooner than waiting for the full 512-col bias add
                    hw = PC // 2
                    for i in range(2):
                        nc.vector.tensor_scalar(
                            kT_sb[:, chn * PC + i * hw:chn * PC + (i + 1) * hw],
                            ps[:, i * hw:(i + 1) * hw],
                            b_sb[:, 0:1], None, ALU.add,
                        )
                else:
                    vt = vstage.tile([128, PC], BF16, tag="vt")
                    nc.vector.tensor_scalar(
                        vt[:], ps[:, :], b_sb[:, 0:1], None, ALU.add
                    )
                    vt_tiles[chn] = vt
            return f

        def v_transp(chn, jls):
            def f():
                vt = vt_tiles[chn]
                for jl in jls:
                    idx = chn * (PC // 128) + jl     # global 128-token tile
                    pt = psing.tile(
                        [128, 128], BF16, tag="single", padded_shape=[128, 512]
                    )
                    nc.tensor.transpose(
                        pt[:, :], vt[:, jl * 128:(jl + 1) * 128], ident[:]
                    )
                    nc.vector.tensor_copy(
                        v_sb[:, idx, :, 0:DH],
                        pt[:].rearrange("p (h c) -> p h c", h=2),
                    )
            return f

        def chunk_items(chn, with_q=True):
            items = [proj_col(chn, "k"), proj_col(chn, "v"),
                     v_transp(chn, (0, 1)), v_transp(chn, (2, 3))]
            if with_q:
                items.append(proj_col(chn, "q"))
            return items

        def chunk_items_split(chn):
            # smaller bursts for the non-deadline-critical b1 chunks
            return [proj_col(chn, "k", 0), proj_col(chn, "k", 1),
                    proj_col(chn, "v", 0), proj_col(chn, "v", 1),
                    v_transp(chn, (0, 1)), v_transp(chn, (2, 3)),
                    proj_col(chn, "q", 0), proj_col(chn, "q", 1)]

        # chunk 0 k/q/v inline (before attention starts). transposes of
        # chunk 0 go at the queue front (PV needs them only from jtx==2 on)
        # so the PE isn't gated on the v-bias DVE op.
        # ALL x chunks prefetched upfront; x0 quartered so the very first
        # k-projection matmul starts as soon as ~128KB lands.
        x_load(0, split=4)()
        x_load(1, split=2)()
        for chn in range(2, NPC):
            x_load(chn)()
        nc.sync.dma_start(bq_sb[:], bq.ap())
        nc.sync.dma_start(bk_sb[:], bk.ap())
        nc.sync.dma_start(bv_sb[:], bv.ap())
        nc.sync.dma_start(wo_sb[:], wo.ap())
        proj_col(0, "k")()
        proj_col(0, "q")()
        proj_col(0, "v")()

        # queue: k-projections lead their chunk group so b0 scores are
        # never key-starved; transposes trail (PV needs them later).
        queue = deque()
        queue.append(v_transp(0, (0, 1)))
        queue.append(v_transp(0, (2, 3)))
        queue.append(proj_col(1, "k"))
        queue.append(proj_col(1, "v"))
        queue.append(proj_col(2, "k"))
        queue.append(v_transp(1, (0, 1)))
        queue.append(proj_col(2, "v"))
        queue.append(proj_col(3, "k"))
        queue.append(v_transp(1, (2, 3)))
        queue.append(proj_col(3, "v"))
        queue.append(v_transp(2, (0, 1)))
        queue.append(v_transp(2, (2, 3)))
        queue.append(proj_col(1, "q"))
        queue.append(v_transp(3, (0, 1)))
        queue.append(v_transp(3, (2, 3)))
        queue.append(proj_col(2, "q"))
        queue.append(proj_col(3, "q"))
        for chn in range(4, NPC):
            queue.extend(chunk_items_split(chn))

        def pop_queue():
            if queue:
                queue.popleft()()

        # --- attention ---
        def make_drain1(b, q0c, LCc, aT_ps):
            # part a (jtx==0): copy the FULL accumulator out of PSUM (DVE).
            # This frees the PSUM bank immediately, so the next chunk's
            # PV (start=True at jtx==PIPE) never waits on the normalize.
            # part b (jtx==2): broadcast -> reciprocal -> normalize, all
            # reading the SBUF copy.
            st = {}

            def fa():
                # den row: equal-base copy; head blocks: PSUM base-0 in,
                # SBUF base h*64 out (up-shift, same as the proven
                # normalize pattern). NEVER shift a base DOWNWARD — custom
                # DVE ops silently read the wrong partitions.
                den = drpool.tile([65, HPC, LCc], BF16, tag="den",
                                  padded_shape=[65, HPC, 512])
                ac = drpool.tile([128, LCc], BF16, tag="ac",
                                 padded_shape=[128, 512])
                nc.vector.tensor_copy(den[64:65, :, :], aT_ps[64:65, :, :])
                for h in range(HPC):
                    nc.vector.tensor_copy(
                        ac[h * DH:(h + 1) * DH, :], aT_ps[0:DH, h, :]
                    )
                st["den"], st["ac"] = den, ac

            def fb():
                den, ac = st["den"], st["ac"]
                rep = psing.tile([128, LCc], F32, tag="single",
                                 padded_shape=[128, 512])
                for h in range(HPC):
                    nc.tensor.matmul(
                        rep[h * DH:(h + 1) * DH, :],
                        ones64[64:65, :],
                        den[64:65, h, :],
                        start=True, stop=True,
                        tile_position=(64, h * DH),
                    )
                rrec = drpool.tile([128, LCc], F32, tag="rrec",
                                   padded_shape=[128, 512])
                nc.vector.reciprocal_approx_fast(rrec[:, :], rep[:, :])
                for h in range(HPC):
                    # h1 on gpsimd (SBUF-only op): halves the serial
                    # normalize latency at chunk boundaries
                    eng = nc.vector if h == 0 else nc.gpsimd
                    eng.tensor_mul(
                        aT_sb[b][h * DH:(h + 1) * DH, q0c:q0c + LCc],
                        ac[h * DH:(h + 1) * DH, :],
                        rrec[h * DH:(h + 1) * DH, :],
                    )
            return fa, fb

        def make_drain2(b, q0c, t, fine=False):
            # output projection for one 128-token tile of this query chunk.
            # fine=True (the very last tile): DMA each 512-col half as soon
            # as its copy lands, shortening the end-of-kernel flush.
            def f():
                tt = q0c // 128 + t
                ot = outpool.tile([128, D], BF16, tag="ot")
                for nch in range(2):
                    po = psing.tile([128, 512], F32, tag="single")
                    nc.tensor.matmul(
                        po[:, :],
                        aT_sb[b][:, tt * 128:(tt + 1) * 128],
                        wo_sb[:, nch * 512:(nch + 1) * 512],
                        start=True, stop=True,
                    )
                    nc.vector.tensor_copy(
                        ot[:, nch * 512:(nch + 1) * 512], po[:, :]
                    )
                    if fine:
                        nc.sync.dma_start(
                            out.ap()[b * Lb + tt * 128:b * Lb + (tt + 1) * 128,
                                     nch * 512:(nch + 1) * 512],
                            ot[:, nch * 512:(nch + 1) * 512],
                        )
                if not fine:
                    nc.sync.dma_start(
                        out.ap()[b * Lb + tt * 128:b * Lb + (tt + 1) * 128, :],
                        ot[:],
                    )
            return f

        # query chunks per batch: b1 tapers so the final (fully exposed)
        # drain chain covers only 128 queries instead of 512.
        def chunks_of(b):
            if Lb < 512:
                return [(i * 128, 128) for i in range(Lb // 128)]
            full = [(i * LC, LC) for i in range(NLC)]
            if b == B - 1:
                return full[:-1] + [(Lb - LC, 384), (Lb - 128, 128)]
            return full

        # software pipeline: scores/exp run PIPE jt-steps ahead of PV, so the
        # previous chunk's drain (spread over jtx 2..10) never starves ACT.
        PIPE = 4
        drain1a = drain1b = None
        drain2_pending = []
        deferred2 = deque()   # b0 outproj drains, parked for the dry phase
        deferred_hold = []    # not yet released: their drain1b hasn't run
        slot = 0
        for b in range(B):
            for (q0c, LCc) in chunks_of(b):
                q0 = b * Lb + q0c
                aT_ps = accpool.tile(
                    [65, HPC, LCc], F32, tag="acc", padded_shape=[65, HPC, 512]
                )
                ex_fifo = deque()
                for jtx in range(NJT + PIPE):
                    if jtx < NJT:
                        # consume projection work every slot, except when a
                        # drain piece already adds PE work to this slot
                        drain_here = (jtx == 2 and drain1b is not None) or \
                                     (jtx in (4, 6, 8, 10) and drain2_pending)
                        if not drain_here:
                            if queue:
                                pop_queue()
                                if slot < 2:
                                    pop_queue()  # front-load chunk-0 transposes
                            elif deferred2:
                                # queue dry: feed the PE parked b0 outproj
                                # work so late slots aren't ACT-bound
                                deferred2.popleft()()
                        slot += 1
                        k0 = b * Lb + jtx * 128
                        sc = scpool.tile(
                            [128, HPC, LCc], F32, tag="sc",
                            padded_shape=[128, HPC, 512],
                        )
                        for h in range(HPC):
                            nc.tensor.matmul(
                                sc[:, h, :],
                                kT_sb[h * DH:(h + 1) * DH, k0:k0 + 128],
                                qT_sb[h * DH:(h + 1) * DH, q0:q0 + LCc],
                                start=True, stop=True,
                                tile_position=(h * DH, 0),
                            )
                        ex = expool.tile([128, HPC, LCc], BF16, tag="ex",
                                         padded_shape=[128, HPC, 512])
                        nc.scalar.activation(ex[:], sc[:], AF.Exp)
                        ex_fifo.append(ex)
                    if jtx == 0 and drain1a is not None:
                        drain1a()
                        drain1a = None
                    if jtx == 2 and drain1b is not None:
                        drain1b()
                        drain1b = None
                        # normalize emitted: its outproj may now be parked
                        deferred2.extend(deferred_hold)
                        deferred_hold.clear()
                    if jtx in (4, 6, 8, 10) and drain2_pending:
                        drain2_pending.pop(0)()
                    if jtx >= PIPE:
                        jt = jtx - PIPE
                        tt = b * NJT + jt             # global 128-token tile
                        ex = ex_fifo.popleft()
                        for h in range(HPC):
                            nc.tensor.matmul(
                                aT_ps[:, h, :],
                                v_sb[:, tt, h, :],
                                ex[:, h, :],
                                start=(jt == 0), stop=(jt == NJT - 1),
                            )
                drain1a, drain1b = make_drain1(b, q0c, LCc, aT_ps)
                is_last = (b == B - 1) and (q0c + LCc == Lb)
                items = [
                    make_drain2(b, q0c, t, fine=is_last and t == LCc // 128 - 1)
                    for t in range(LCc // 128)
                ]
                if b == 0:
                    deferred_hold.extend(items)
                    drain2_pending = []
                else:
                    drain2_pending = items
        drain1a()
        drain1b()
        deferred2.extend(deferred_hold)
        while deferred2:
            deferred2.popleft()()
        for f in drain2_pending:
            f()
        while queue:
            pop_queue()

    nc.compile()
    return nc


_NC_CACHE = {}


def _get_nc(Lb=L):
    if Lb not in _NC_CACHE:
        _NC_CACHE[Lb] = build(Lb)
    return _NC_CACHE[Lb]


def make_in_maps(x, Wq, bq, Wk, bk, Wv, bv, Wo, bo, Lb=L):
    import ml_dtypes
    bf16 = ml_dtypes.bfloat16
    s = np.float32(DH ** (-0.25))
    BLb = B * Lb
    PC = min(512, BLb)
    NPC = BLb // PC
    # [128, NPC, KT, PC]: partition-major, chunk-contiguous
    xT = np.ascontiguousarray(
        np.asarray(x, np.float32).reshape(NPC, PC, KT, 128)
        .transpose(3, 0, 2, 1)
    ).astype(bf16)

    def wprep(w):   # [D, DHC] -> [128, KT, DHC]
        return np.ascontiguousarray(
            w.reshape(KT, 128, -1).transpose(1, 0, 2).astype(bf16)
        )

    Wq, Wk, Wv, Wo = (np.asarray(a, np.float32) for a in (Wq, Wk, Wv, Wo))
    bq, bk, bv = (np.asarray(a, np.float32) for a in (bq, bk, bv))
    in_maps = []
    for c in range(NCORES):
        hs = slice(c * DHC, (c + 1) * DHC)
        in_maps.append({
            "xT": xT,
            "wq": wprep(Wq[:, hs] * s),
            "wk": wprep(Wk[:, hs] * s),
            "wv": wprep(Wv[:, hs]),
            "wo": np.ascontiguousarray(Wo[hs, :].astype(bf16)),
            "bq": np.ascontiguousarray((bq[hs] * s).reshape(DHC, 1)),
            "bk": np.ascontiguousarray((bk[hs] * s).reshape(DHC, 1)),
            "bv": np.ascontiguousarray(bv[hs].reshape(DHC, 1)),
        })
    return in_maps


def kernel(x, Wq, bq, Wk, bk, Wv, bv, Wo, bo, **run_kwargs):
    x = np.asarray(x, np.float32)
    nc = _get_nc(L)
    in_maps = make_in_maps(x, Wq, bq, Wk, bk, Wv, bv, Wo, bo, L)
    res = bass_utils.run_bass_kernel_spmd(nc, in_maps, list(range(NCORES)), **run_kwargs)
    acc = np.zeros((B * L, D), np.float32)
    for r in res.results:
        acc += np.asarray(r["out"], np.float32)
    acc += np.asarray(bo, np.float32)[None, :]
    out = acc.reshape(B, L, D)
    kernel.last_results = res
    return out



# revision 31
# speedup vs baseline: 1.0644x; 1.0644x over previous
"""Trainium2 Bass kernel for nn_AttentionBlock (B=2, L=2048, D=1024, H=16).

Sharding: tensor-parallel over heads. Each of 8 cores computes 2 heads:
Wq/Wk/Wv column-sharded, Wo row-sharded; host sums the 8 partial outputs.

v3: all-x upfront multi-queue prefetch, bf16 ex/v PV datapath, full-PSUM
early drain copy (kills chunk-boundary WAR stalls), tapered final chunks
(512,512,512,384,128 on b1) to shrink the exposed tail drain.

v2 structure (per core, bf16 datapath, fp32 PSUM):
  - v stored [token, head*64+d] per 128-token block (PE transpose, no ones col)
  - scores: 2 heads row-packed (K=64) in one PE slot
  - PV: 2 heads col-packed (M=64 at cols 0/64) in one PE slot
  - softmax denominators: ones[128,64] lhsT broadcast matmuls, col-packed,
    accumulating [128,LC] PSUM that directly IS the per-head-row denominator
    layout -> reciprocal_approx_fast -> normalize fused into the PSUM drain
  - projections interleaved into attention slots (work queue) so the PE and
    ACT engines stay dense; exp table preloaded via a dummy activation
"""
import numpy as np
from contextlib import ExitStack
from collections import deque

import concourse.bacc as bacc
import concourse.tile as tile
import concourse.mybir as mybir
from concourse import bass_utils
from concourse.masks import make_identity

F32 = mybir.dt.float32
F32R = mybir.dt.float32r
BF16 = mybir.dt.bfloat16
AF = mybir.ActivationFunctionType
ALU = mybir.AluOpType

B, L, D, H, DH = 2, 2048, 1024, 16, 64
NCORES = 8
HPC = H // NCORES       # heads per core = 2
DHC = HPC * DH          # 128
KT = D // 128           # 8 k-tiles over the contraction dim


def build(Lb=L, debug=False):
    BLb = B * Lb
    NJT = Lb // 128            # key tiles per batch = 16
    LC = min(512, Lb)          # query-chunk width
    NLC = Lb // LC             # query chunks per batch = 4
    PC = min(512, BLb)         # projection chunk width
    NPC = BLb // PC            # projection chunks (global) = 8
    NTT = BLb // 128           # 128-token tiles (global) = 32

    nc = bacc.Bacc("TRN2", target_bir_lowering=False, debug=debug, num_devices=8)

    # host supplies SBUF-layout (partition-major, chunk-contiguous) arrays so
    # every DMA is contiguous per partition (fast descriptor gen + transfer)
    NPC_ = BLb // min(512, BLb)
    xT = nc.dram_tensor("xT", [128, NPC_, KT, min(512, BLb)], BF16,
                        kind="ExternalInput")
    wq = nc.dram_tensor("wq", [128, KT, DHC], BF16, kind="ExternalInput")
    wk = nc.dram_tensor("wk", [128, KT, DHC], BF16, kind="ExternalInput")
    wv = nc.dram_tensor("wv", [128, KT, DHC], BF16, kind="ExternalInput")
    wo = nc.dram_tensor("wo", [DHC, D], BF16, kind="ExternalInput")
    bq = nc.dram_tensor("bq", [DHC, 1], F32, kind="ExternalInput")
    bk = nc.dram_tensor("bk", [DHC, 1], F32, kind="ExternalInput")
    bv = nc.dram_tensor("bv", [DHC, 1], F32, kind="ExternalInput")
    out = nc.dram_tensor("out", [BLb, D], BF16, kind="ExternalOutput")

    xT_v = xT.ap()                                          # [128, NPC, KT, PC]
    wq_v = wq.ap()
    wk_v = wk.ap()
    wv_v = wv.ap()

    with tile.TileContext(nc) as tc, ExitStack() as ctx:
        # --- pools ---
        persist = ctx.enter_context(tc.tile_pool(name="persist", bufs=1))
        xpool = ctx.enter_context(tc.tile_pool(name="xchunk", bufs=NPC))
        vstage = ctx.enter_context(tc.tile_pool(name="vstage", bufs=2))
        expool = ctx.enter_context(tc.tile_pool(name="expool", bufs=5))
        drpool = ctx.enter_context(tc.tile_pool(name="drpool", bufs=2))
        outpool = ctx.enter_context(tc.tile_pool(name="outpool", bufs=3))
        # PSUM: sc 2banks x2 + aT 1 + den 1 + single 1x2 = 8 banks
        scpool = ctx.enter_context(tc.tile_pool(name="scpool", bufs=2, space="PSUM"))
        accpool = ctx.enter_context(tc.tile_pool(name="accpool", bufs=1, space="PSUM"))
        psing = ctx.enter_context(tc.tile_pool(name="psing", bufs=2, space="PSUM"))

        # --- persistent tiles ---
        qT_sb = persist.tile([128, BLb], BF16, tag="qT")
        kT_sb = persist.tile([128, BLb], BF16, tag="kT")
        v_sb = persist.tile([128, NTT, 2, 65], BF16, tag="v")
        aT_sb = [
            persist.tile([128, Lb], BF16, tag=f"aT{b}", name=f"aT{b}")
            for b in range(B)
        ]
        wq_sb = persist.tile([128, KT, DHC], BF16, tag="wq")
        wk_sb = persist.tile([128, KT, DHC], BF16, tag="wk")
        wv_sb = persist.tile([128, KT, DHC], BF16, tag="wv")
        wo_sb = persist.tile([DHC, D], BF16, tag="wo")
        bq_sb = persist.tile([DHC, 1], F32, tag="bq")
        bk_sb = persist.tile([DHC, 1], F32, tag="bk")
        bv_sb = persist.tile([DHC, 1], F32, tag="bv")
        ident = persist.tile([128, 128], BF16, tag="ident")
        ones64 = persist.tile([65, DH], BF16, tag="ones64")   # row 64 used
        dumm = persist.tile([1, 2], F32, tag="dumm")

        xt_tiles = {}
        vt_tiles = {}

        # --- prologue: weight loads, constants, ACT exp-table preload ---
        # wk/x0 split in halves so the first k-col matmuls start as soon as
        # the first half lands (~2.5us) instead of after the full loads
        for i in range(4):
            w4 = KT // 4
            nc.sync.dma_start(wk_sb[:, i * w4:(i + 1) * w4, :],
                              wk_v[:, i * w4:(i + 1) * w4, :])
        nc.sync.dma_start(wv_sb[:], wv_v)
        nc.sync.dma_start(wq_sb[:], wq_v)
        make_identity(nc, ident[:])
        nc.vector.memset(ones64[:], 1.0)
        nc.vector.memset(dumm[:], 0.0)
        nc.scalar.activation(dumm[:], dumm[:], AF.Exp)  # load exp table early
        # ones-augment columns of v (denominator trick)
        nc.vector.memset(v_sb[:, :, :, 64:65], 1.0)

        # --- projection work items ---
        def x_load(chn, split=1):
            def f():
                xt = xpool.tile([128, KT, PC], BF16, tag="xt")
                # scalar-engine HWDGE queue: don't serialize behind the
                # sync-engine weight/output DMA issue stream. split=n
                # pieces let the first kt slices land sooner.
                w = KT // split
                for i in range(split):
                    nc.scalar.dma_start(
                        xt[:, i * w:(i + 1) * w, :],
                        xT_v[:, chn, i * w:(i + 1) * w, :],
                    )
                xt_tiles[chn] = xt
            return f

        ps_tiles = {}

        def proj_col(chn, which, half=None):
            # half=None: full 8-kt column; half=0/1: split into two 4-kt
            # bursts (smaller PE bursts keep ACT fed between slots)
            def f():
                xt = xt_tiles[chn]
                w_sb, b_sb = {
                    "q": (wq_sb, bq_sb), "k": (wk_sb, bk_sb), "v": (wv_sb, bv_sb)
                }[which]
                if half in (None, 0):
                    ps_tiles[(chn, which)] = psing.tile(
                        [128, PC], F32, tag="single", name=f"ps_{chn}_{which}"
                    )
                ps = ps_tiles[(chn, which)]
                kts = range(KT) if half is None else \
                    range(half * KT // 2, (half + 1) * KT // 2)
                for kt in kts:
                    nc.tensor.matmul(
                        ps[:, :], w_sb[:, kt, :], xt[:, kt, :],
                        start=(kt == 0), stop=(kt == KT - 1),
                    )
                if half == 0:
                    return
                if which == "q":
                    nc.vector.tensor_scalar(
                        qT_sb[:, chn * PC:(chn + 1) * PC], ps[:, :],
                        b_sb[:, 0:1], None, ALU.add,
                    )
                elif which == "k":
                    # split: scores of the first key-tiles unblock ~500ns
                    # sooner than waiting for the full 512-col bias add
                    hw = PC // 2
                    for i in range(2):
                        nc.vector.tensor_scalar(
                            kT_sb[:, chn * PC + i * hw:chn * PC + (i + 1) * hw],
                            ps[:, i * hw:(i + 1) * hw],
                            b_sb[:, 0:1], None, ALU.add,
                        )
                else:
                    vt = vstage.tile([128, PC], BF16, tag="vt")
                    nc.vector.tensor_scalar(
                        vt[:], ps[:, :], b_sb[:, 0:1], None, ALU.add
                    )
                    vt_tiles[chn] = vt
            return f

        def v_transp(chn, jls):
            def f():
                vt = vt_tiles[chn]
                for jl in jls:
                    idx = chn * (PC // 128) + jl     # global 128-token tile
                    pt = psing.tile(
                        [128, 128], BF16, tag="single", padded_shape=[128, 512]
                    )
                    nc.tensor.transpose(
                        pt[:, :], vt[:, jl * 128:(jl + 1) * 128], ident[:]
                    )
                    nc.vector.tensor_copy(
                        v_sb[:, idx, :, 0:DH],
                        pt[:].rearrange("p (h c) -> p h c", h=2),
                    )
            return f

        def chunk_items(chn, with_q=True):
            items = [proj_col(chn, "k"), proj_col(chn, "v"),
                     v_transp(chn, (0, 1)), v_transp(chn, (2, 3))]
            if with_q:
                items.append(proj_col(chn, "q"))
            return items

        def chunk_items_split(chn):
            # smaller bursts for the non-deadline-critical b1 chunks
            return [proj_col(chn, "k", 0), proj_col(chn, "k", 1),
                    proj_col(chn, "v", 0), proj_col(chn, "v", 1),
                    v_transp(chn, (0, 1)), v_transp(chn, (2, 3)),
                    proj_col(chn, "q", 0), proj_col(chn, "q", 1)]

        # chunk 0 k/q/v inline (before attention starts). transposes of
        # chunk 0 go at the queue front (PV needs them only from jtx==2 on)
        # so the PE isn't gated on the v-bias DVE op.
        # ALL x chunks prefetched upfront; x0 quartered so the very first
        # k-projection matmul starts as soon as ~128KB lands.
        x_load(0, split=4)()
        x_load(1, split=2)()
        for chn in range(2, NPC):
            x_load(chn)()
        nc.sync.dma_start(bq_sb[:], bq.ap())
        nc.sync.dma_start(bk_sb[:], bk.ap())
        nc.sync.dma_start(bv_sb[:], bv.ap())
        nc.sync.dma_start(wo_sb[:], wo.ap())
        proj_col(0, "k")()
        proj_col(0, "q")()
        proj_col(0, "v")()

        # queue: k-projections lead their chunk group so b0 scores are
        # never key-starved; transposes trail (PV needs them later).
        queue = deque()
        queue.append(v_transp(0, (0, 1)))
        queue.append(v_transp(0, (2, 3)))
        queue.append(proj_col(1, "k"))
        queue.append(proj_col(1, "v"))
        queue.append(proj_col(2, "k"))
        queue.append(v_transp(1, (0, 1)))
        queue.append(proj_col(2, "v"))
        queue.append(proj_col(3, "k"))
        queue.append(v_transp(1, (2, 3)))
        queue.append(proj_col(3, "v"))
        queue.append(v_transp(2, (0, 1)))
        queue.append(v_transp(2, (2, 3)))
        queue.append(proj_col(1, "q"))
        queue.append(v_transp(3, (0, 1)))
        queue.append(v_transp(3, (2, 3)))
        queue.append(proj_col(2, "q"))
        queue.append(proj_col(3, "q"))
        for chn in range(4, NPC):
            queue.extend(chunk_items_split(chn))

        def pop_queue():
            if queue:
                queue.popleft()()

        # --- attention ---
        def make_drain1(b, q0c, LCc, aT_ps):
            # part a (jtx==0): copy the FULL accumulator out of PSUM (DVE).
            # This frees the PSUM bank immediately, so the next chunk's
            # PV (start=True at jtx==PIPE) never waits on the normalize.
            # part b (jtx==2): broadcast -> reciprocal -> normalize, all
            # reading the SBUF copy.
            st = {}

            def fa():
                # den row: equal-base copy; head blocks: PSUM base-0 in,
                # SBUF base h*64 out (up-shift, same as the proven
                # normalize pattern). NEVER shift a base DOWNWARD — custom
                # DVE ops silently read the wrong partitions.
                den = drpool.tile([65, HPC, LCc], BF16, tag="den",
                                  padded_shape=[65, HPC, 512])
                ac = drpool.tile([128, LCc], BF16, tag="ac",
                                 padded_shape=[128, 512])
                nc.vector.tensor_copy(den[64:65, :, :], aT_ps[64:65, :, :])
                for h in range(HPC):
                    nc.vector.tensor_copy(
                        ac[h * DH:(h + 1) * DH, :], aT_ps[0:DH, h, :]
                    )
                st["den"], st["ac"] = den, ac

            def fb():
                den, ac = st["den"], st["ac"]
                rep = psing.tile([128, LCc], F32, tag="single",
                                 padded_shape=[128, 512])
                for h in range(HPC):
                    nc.tensor.matmul(
                        rep[h * DH:(h + 1) * DH, :],
                        ones64[64:65, :],
                        den[64:65, h, :],
                        start=True, stop=True,
                        tile_position=(64, h * DH),
                    )
                rrec = drpool.tile([128, LCc], F32, tag="rrec",
                                   padded_shape=[128, 512])
                nc.vector.reciprocal_approx_fast(rrec[:, :], rep[:, :])
                for h in range(HPC):
                    nc.vector.tensor_mul(
                        aT_sb[b][h * DH:(h + 1) * DH, q0c:q0c + LCc],
                        ac[h * DH:(h + 1) * DH, :],
                        rrec[h * DH:(h + 1) * DH, :],
                    )
            return fa, fb

        def make_drain2(b, q0c, t, fine=False):
            # output projection for one 128-token tile of this query chunk.
            # fine=True (the very last tile): DMA each 512-col half as soon
            # as its copy lands, shortening the end-of-kernel flush.
            def f():
                tt = q0c // 128 + t
                ot = outpool.tile([128, D], BF16, tag="ot")
                for nch in range(2):
                    po = psing.tile([128, 512], F32, tag="single")
                    nc.tensor.matmul(
                        po[:, :],
                        aT_sb[b][:, tt * 128:(tt + 1) * 128],
                        wo_sb[:, nch * 512:(nch + 1) * 512],
                        start=True, stop=True,
                    )
                    nc.vector.tensor_copy(
                        ot[:, nch * 512:(nch + 1) * 512], po[:, :]
                    )
                    if fine:
                        nc.sync.dma_start(
                            out.ap()[b * Lb + tt * 128:b * Lb + (tt + 1) * 128,
                                     nch * 512:(nch + 1) * 512],
                            ot[:, nch * 512:(nch + 1) * 512],
                        )
                if not fine:
                    nc.sync.dma_start(
                        out.ap()[b * Lb + tt * 128:b * Lb + (tt + 1) * 128, :],
                        ot[:],
                    )
            return f

        # query chunks per batch: b1 tapers so the final (fully exposed)
        # drain chain covers only 128 queries instead of 512.
        def chunks_of(b):
            if Lb < 512:
                return [(i * 128, 128) for i in range(Lb // 128)]
            full = [(i * LC, LC) for i in range(NLC)]
            if b == B - 1:
                return full[:-1] + [(Lb - LC, 384), (Lb - 128, 128)]
            return full

        # software pipeline: scores/exp run PIPE jt-steps ahead of PV, so the
        # previous chunk's drain (spread over jtx 2..10) never starves ACT.
        PIPE = 4
        drain1a = drain1b = None
        drain2_pending = []
        deferred2 = deque()   # b0 outproj drains, parked for the dry phase
        deferred_hold = []    # not yet released: their drain1b hasn't run
        slot = 0
        for b in range(B):
            for (q0c, LCc) in chunks_of(b):
                q0 = b * Lb + q0c
                aT_ps = accpool.tile(
                    [65, HPC, LCc], F32, tag="acc", padded_shape=[65, HPC, 512]
                )
                ex_fifo = deque()
                for jtx in range(NJT + PIPE):
                    if jtx < NJT:
                        # consume projection work every slot, except when a
                        # drain piece already adds PE work to this slot
                        drain_here = (jtx == 2 and drain1b is not None) or \
                                     (jtx in (4, 6, 8, 10) and drain2_pending)
                        if not drain_here:
                            if queue:
                                pop_queue()
                                if slot < 2:
                                    pop_queue()  # front-load chunk-0 transposes
                            elif deferred2:
                                # queue dry: feed the PE parked b0 outproj
                                # work so late slots aren't ACT-bound
                                deferred2.popleft()()
                        slot += 1
                        k0 = b * Lb + jtx * 128
                        sc = scpool.tile(
                            [128, HPC, LCc], F32, tag="sc",
                            padded_shape=[128, HPC, 512],
                        )
                        for h in range(HPC):
                            nc.tensor.matmul(
                                sc[:, h, :],
                                kT_sb[h * DH:(h + 1) * DH, k0:k0 + 128],
                                qT_sb[h * DH:(h + 1) * DH, q0:q0 + LCc],
                                start=True, stop=True,
                                tile_position=(h * DH, 0),
                            )
                        ex = expool.tile([128, HPC, LCc], BF16, tag="ex",
                                         padded_shape=[128, HPC, 512])
                        nc.scalar.activation(ex[:], sc[:], AF.Exp)
                        ex_fifo.append(ex)
                    if jtx == 0 and drain1a is not None:
                        drain1a()
                        drain1a = None
                    if jtx == 2 and drain1b is not None:
                        drain1b()
                        drain1b = None
                        # normalize emitted: its outproj may now be parked
                        deferred2.extend(deferred_hold)
                        deferred_hold.clear()
                    if jtx in (4, 6, 8, 10) and drain2_pending:
                        drain2_pending.pop(0)()
                    if jtx >= PIPE:
                        jt = jtx - PIPE
                        tt = b * NJT + jt             # global 128-token tile
                        ex = ex_fifo.popleft()
                        for h in range(HPC):
                            nc.tensor.matmul(
                                aT_ps[:, h, :],
                                v_sb[:, tt, h, :],
                                ex[:, h, :],
                                start=(jt == 0), stop=(jt == NJT - 1),
                            )
                drain1a, drain1b = make_drain1(b, q0c, LCc, aT_ps)
                is_last = (b == B - 1) and (q0c + LCc == Lb)
                items = [
                    make_drain2(b, q0c, t, fine=is_last and t == LCc // 128 - 1)
                    for t in range(LCc // 128)
                ]
                if b == 0:
                    deferred_hold.extend(items)
                    drain2_pending = []
                else:
                    drain2_pending = items
        drain1a()
        drain1b()
        deferred2.extend(deferred_hold)
        while deferred2:
            deferred2.popleft()()
        for f in drain2_pending:
            f()
        while queue:
            pop_queue()

    nc.compile()
    return nc


_NC_CACHE = {}


def _get_nc(Lb=L):
    if Lb not in _NC_CACHE:
        _NC_CACHE[Lb] = build(Lb)
    return _NC_CACHE[Lb]


def make_in_maps(x, Wq, bq, Wk, bk, Wv, bv, Wo, bo, Lb=L):
    import ml_dtypes
    bf16 = ml_dtypes.bfloat16
    s = np.float32(DH ** (-0.25))
    BLb = B * Lb
    PC = min(512, BLb)
    NPC = BLb // PC
    # [128, NPC, KT, PC]: partition-major, chunk-contiguous
    xT = np.ascontiguousarray(
        np.asarray(x, np.float32).reshape(NPC, PC, KT, 128)
        .transpose(3, 0, 2, 1)
    ).astype(bf16)

    def wprep(w):   # [D, DHC] -> [128, KT, DHC]
        return np.ascontiguousarray(
            w.reshape(KT, 128, -1).transpose(1, 0, 2).astype(bf16)
        )

    Wq, Wk, Wv, Wo = (np.asarray(a, np.float32) for a in (Wq, Wk, Wv, Wo))
    bq, bk, bv = (np.asarray(a, np.float32) for a in (bq, bk, bv))
    in_maps = []
    for c in range(NCORES):
        hs = slice(c * DHC, (c + 1) * DHC)
        in_maps.append({
            "xT": xT,
            "wq": wprep(Wq[:, hs] * s),
            "wk": wprep(Wk[:, hs] * s),
            "wv": wprep(Wv[:, hs]),
            "wo": np.ascontiguousarray(Wo[hs, :].astype(bf16)),
            "bq": np.ascontiguousarray((bq[hs] * s).reshape(DHC, 1)),
            "bk": np.ascontiguousarray((bk[hs] * s).reshape(DHC, 1)),
            "bv": np.ascontiguousarray(bv[hs].reshape(DHC, 1)),
        })
    return in_maps


def kernel(x, Wq, bq, Wk, bk, Wv, bv, Wo, bo, **run_kwargs):
    x = np.asarray(x, np.float32)
    nc = _get_nc(L)
    in_maps = make_in_maps(x, Wq, bq, Wk, bk, Wv, bv, Wo, bo, L)
    res = bass_utils.run_bass_kernel_spmd(nc, in_maps, list(range(NCORES)), **run_kwargs)
    acc = np.zeros((B * L, D), np.float32)
    for r in res.results:
        acc += np.asarray(r["out"], np.float32)
    acc += np.asarray(bo, np.float32)[None, :]
    out = acc.reshape(B, L, D)
    kernel.last_results = res
    return out



# revision 32
# speedup vs baseline: 1.0840x; 1.0184x over previous
"""Trainium2 Bass kernel for nn_AttentionBlock (B=2, L=2048, D=1024, H=16).

Sharding: tensor-parallel over heads. Each of 8 cores computes 2 heads:
Wq/Wk/Wv column-sharded, Wo row-sharded; host sums the 8 partial outputs.

v7 (final): upfront x prefetch (x0 quartered, wk quartered, rest whole)
so projections are never DMA-gated; bf16 ex/v PV datapath (fp32r was
single-pass-bf16 anyway, bf16 keeps small-N chunks full rate); early
full-PSUM drain copy at chunk boundaries (base-aligned — custom DVE ops
silently corrupt on downward partition-base shifts); k-bias adds split
in halves to unblock scores sooner; k-first queue order; PIPE=4;
tapered final chunks (512,512,512,384,128 on b1) + per-half final out
DMA to shrink the exposed tail; b0 outproj drains deferred into the
late ACT-bound phase (released only after their chunk's normalize is
emitted — earlier emission would read unwritten aT rows).

v2 structure (per core, bf16 datapath, fp32 PSUM):
  - v stored [token, head*64+d] per 128-token block (PE transpose, no ones col)
  - scores: 2 heads row-packed (K=64) in one PE slot
  - PV: 2 heads col-packed (M=64 at cols 0/64) in one PE slot
  - softmax denominators: ones[128,64] lhsT broadcast matmuls, col-packed,
    accumulating [128,LC] PSUM that directly IS the per-head-row denominator
    layout -> reciprocal_approx_fast -> normalize fused into the PSUM drain
  - projections interleaved into attention slots (work queue) so the PE and
    ACT engines stay dense; exp table preloaded via a dummy activation
"""
import numpy as np
from contextlib import ExitStack
from collections import deque

import concourse.bacc as bacc
import concourse.tile as tile
import concourse.mybir as mybir
from concourse import bass_utils
from concourse.masks import make_identity

F32 = mybir.dt.float32
F32R = mybir.dt.float32r
BF16 = mybir.dt.bfloat16
AF = mybir.ActivationFunctionType
ALU = mybir.AluOpType

B, L, D, H, DH = 2, 2048, 1024, 16, 64
NCORES = 8
HPC = H // NCORES       # heads per core = 2
DHC = HPC * DH          # 128
KT = D // 128           # 8 k-tiles over the contraction dim


def build(Lb=L, debug=False):
    BLb = B * Lb
    NJT = Lb // 128            # key tiles per batch = 16
    LC = min(512, Lb)          # query-chunk width
    NLC = Lb // LC             # query chunks per batch = 4
    PC = min(512, BLb)         # projection chunk width
    NPC = BLb // PC            # projection chunks (global) = 8
    NTT = BLb // 128           # 128-token tiles (global) = 32

    nc = bacc.Bacc("TRN2", target_bir_lowering=False, debug=debug, num_devices=8)

    # host supplies SBUF-layout (partition-major, chunk-contiguous) arrays so
    # every DMA is contiguous per partition (fast descriptor gen + transfer)
    NPC_ = BLb // min(512, BLb)
    xT = nc.dram_tensor("xT", [128, NPC_, KT, min(512, BLb)], BF16,
                        kind="ExternalInput")
    wq = nc.dram_tensor("wq", [128, KT, DHC], BF16, kind="ExternalInput")
    wk = nc.dram_tensor("wk", [128, KT, DHC], BF16, kind="ExternalInput")
    wv = nc.dram_tensor("wv", [128, KT, DHC], BF16, kind="ExternalInput")
    wo = nc.dram_tensor("wo", [DHC, D], BF16, kind="ExternalInput")
    bq = nc.dram_tensor("bq", [DHC, 1], F32, kind="ExternalInput")
    bk = nc.dram_tensor("bk", [DHC, 1], F32, kind="ExternalInput")
    bv = nc.dram_tensor("bv", [DHC, 1], F32, kind="ExternalInput")
    out = nc.dram_tensor("out", [BLb, D], BF16, kind="ExternalOutput")

    xT_v = xT.ap()                                          # [128, NPC, KT, PC]
    wq_v = wq.ap()
    wk_v = wk.ap()
    wv_v = wv.ap()

    with tile.TileContext(nc) as tc, ExitStack() as ctx:
        # --- pools ---
        persist = ctx.enter_context(tc.tile_pool(name="persist", bufs=1))
        xpool = ctx.enter_context(tc.tile_pool(name="xchunk", bufs=NPC))
        vstage = ctx.enter_context(tc.tile_pool(name="vstage", bufs=2))
        expool = ctx.enter_context(tc.tile_pool(name="expool", bufs=5))
        drpool = ctx.enter_context(tc.tile_pool(name="drpool", bufs=2))
        outpool = ctx.enter_context(tc.tile_pool(name="outpool", bufs=3))
        # PSUM: sc 2banks x2 + aT 1 + den 1 + single 1x2 = 8 banks
        scpool = ctx.enter_context(tc.tile_pool(name="scpool", bufs=2, space="PSUM"))
        accpool = ctx.enter_context(tc.tile_pool(name="accpool", bufs=1, space="PSUM"))
        psing = ctx.enter_context(tc.tile_pool(name="psing", bufs=2, space="PSUM"))

        # --- persistent tiles ---
        qT_sb = persist.tile([128, BLb], BF16, tag="qT")
        kT_sb = persist.tile([128, BLb], BF16, tag="kT")
        v_sb = persist.tile([128, NTT, 2, 65], BF16, tag="v")
        aT_sb = [
            persist.tile([128, Lb], BF16, tag=f"aT{b}", name=f"aT{b}")
            for b in range(B)
        ]
        wq_sb = persist.tile([128, KT, DHC], BF16, tag="wq")
        wk_sb = persist.tile([128, KT, DHC], BF16, tag="wk")
        wv_sb = persist.tile([128, KT, DHC], BF16, tag="wv")
        wo_sb = persist.tile([DHC, D], BF16, tag="wo")
        bq_sb = persist.tile([DHC, 1], F32, tag="bq")
        bk_sb = persist.tile([DHC, 1], F32, tag="bk")
        bv_sb = persist.tile([DHC, 1], F32, tag="bv")
        ident = persist.tile([128, 128], BF16, tag="ident")
        ones64 = persist.tile([65, DH], BF16, tag="ones64")   # row 64 used
        dumm = persist.tile([1, 2], F32, tag="dumm")

        xt_tiles = {}
        vt_tiles = {}

        # --- prologue: weight loads, constants, ACT exp-table preload ---
        # wk/x0 split in halves so the first k-col matmuls start as soon as
        # the first half lands (~2.5us) instead of after the full loads
        for i in range(4):
            w4 = KT // 4
            nc.sync.dma_start(wk_sb[:, i * w4:(i + 1) * w4, :],
                              wk_v[:, i * w4:(i + 1) * w4, :])
        nc.sync.dma_start(wv_sb[:], wv_v)
        nc.sync.dma_start(wq_sb[:], wq_v)
        make_identity(nc, ident[:])
        nc.vector.memset(ones64[:], 1.0)
        nc.vector.memset(dumm[:], 0.0)
        nc.scalar.activation(dumm[:], dumm[:], AF.Exp)  # load exp table early
        # ones-augment columns of v (denominator trick)
        nc.vector.memset(v_sb[:, :, :, 64:65], 1.0)

        # --- projection work items ---
        def x_load(chn, split=1):
            def f():
                xt = xpool.tile([128, KT, PC], BF16, tag="xt")
                # scalar-engine HWDGE queue: don't serialize behind the
                # sync-engine weight/output DMA issue stream. split=n
                # pieces let the first kt slices land sooner.
                w = KT // split
                for i in range(split):
                    nc.scalar.dma_start(
                        xt[:, i * w:(i + 1) * w, :],
                        xT_v[:, chn, i * w:(i + 1) * w, :],
                    )
                xt_tiles[chn] = xt
            return f

        ps_tiles = {}

        def proj_col(chn, which, half=None):
            # half=None: full 8-kt column; half=0/1: split into two 4-kt
            # bursts (smaller PE bursts keep ACT fed between slots)
            def f():
                xt = xt_tiles[chn]
                w_sb, b_sb = {
                    "q": (wq_sb, bq_sb), "k": (wk_sb, bk_sb), "v": (wv_sb, bv_sb)
                }[which]
                if half in (None, 0):
                    ps_tiles[(chn, which)] = psing.tile(
                        [128, PC], F32, tag="single", name=f"ps_{chn}_{which}"
                    )
                ps = ps_tiles[(chn, which)]
                kts = range(KT) if half is None else \
                    range(half * KT // 2, (half + 1) * KT // 2)
                for kt in kts:
                    nc.tensor.matmul(
                        ps[:, :], w_sb[:, kt, :], xt[:, kt, :],
                        start=(kt == 0), stop=(kt == KT - 1),
                    )
                if half == 0:
                    return
                if which == "q":
                    nc.vector.tensor_scalar(
                        qT_sb[:, chn * PC:(chn + 1) * PC], ps[:, :],
                        b_sb[:, 0:1], None, ALU.add,
                    )
                elif which == "k":
                    # split: scores of the first key-tiles unblock ~500ns
                    # sooner than waiting for the full 512-col bias add
                    hw = PC // 2
                    for i in range(2):
                        nc.vector.tensor_scalar(
                            kT_sb[:, chn * PC + i * hw:chn * PC + (i + 1) * hw],
                            ps[:, i * hw:(i + 1) * hw],
                            b_sb[:, 0:1], None, ALU.add,
                        )
                else:
                    vt = vstage.tile([128, PC], BF16, tag="vt")
                    nc.vector.tensor_scalar(
                        vt[:], ps[:, :], b_sb[:, 0:1], None, ALU.add
                    )
                    vt_tiles[chn] = vt
            return f

        def v_transp(chn, jls):
            def f():
                vt = vt_tiles[chn]
                for jl in jls:
                    idx = chn * (PC // 128) + jl     # global 128-token tile
                    pt = psing.tile(
                        [128, 128], BF16, tag="single", padded_shape=[128, 512]
                    )
                    nc.tensor.transpose(
                        pt[:, :], vt[:, jl * 128:(jl + 1) * 128], ident[:]
                    )
                    nc.vector.tensor_copy(
                        v_sb[:, idx, :, 0:DH],
                        pt[:].rearrange("p (h c) -> p h c", h=2),
                    )
            return f

        def chunk_items(chn, with_q=True):
            items = [proj_col(chn, "k"), proj_col(chn, "v"),
                     v_transp(chn, (0, 1)), v_transp(chn, (2, 3))]
            if with_q:
                items.append(proj_col(chn, "q"))
            return items

        def chunk_items_split(chn):
            # smaller bursts for the non-deadline-critical b1 chunks
            return [proj_col(chn, "k", 0), proj_col(chn, "k", 1),
                    proj_col(chn, "v", 0), proj_col(chn, "v", 1),
                    v_transp(chn, (0, 1)), v_transp(chn, (2, 3)),
                    proj_col(chn, "q", 0), proj_col(chn, "q", 1)]

        # chunk 0 k/q/v inline (before attention starts). transposes of
        # chunk 0 go at the queue front (PV needs them only from jtx==2 on)
        # so the PE isn't gated on the v-bias DVE op.
        # ALL x chunks prefetched upfront; x0 quartered so the very first
        # k-projection matmul starts as soon as ~128KB lands.
        x_load(0, split=4)()
        x_load(1, split=2)()
        for chn in range(2, NPC):
            x_load(chn)()
        nc.sync.dma_start(bq_sb[:], bq.ap())
        nc.sync.dma_start(bk_sb[:], bk.ap())
        nc.sync.dma_start(bv_sb[:], bv.ap())
        nc.sync.dma_start(wo_sb[:], wo.ap())
        proj_col(0, "k")()
        proj_col(0, "q")()
        proj_col(0, "v")()

        # queue: k-projections lead their chunk group so b0 scores are
        # never key-starved; transposes trail (PV needs them later).
        queue = deque()
        queue.append(v_transp(0, (0, 1)))
        queue.append(v_transp(0, (2, 3)))
        queue.append(proj_col(1, "k"))
        queue.append(proj_col(1, "v"))
        queue.append(proj_col(2, "k"))
        queue.append(v_transp(1, (0, 1)))
        queue.append(proj_col(2, "v"))
        queue.append(proj_col(3, "k"))
        queue.append(v_transp(1, (2, 3)))
        queue.append(proj_col(3, "v"))
        queue.append(v_transp(2, (0, 1)))
        queue.append(v_transp(2, (2, 3)))
        queue.append(proj_col(1, "q"))
        queue.append(v_transp(3, (0, 1)))
        queue.append(v_transp(3, (2, 3)))
        queue.append(proj_col(2, "q"))
        queue.append(proj_col(3, "q"))
        for chn in range(4, NPC):
            queue.extend(chunk_items_split(chn))

        def pop_queue():
            if queue:
                queue.popleft()()

        # --- attention ---
        def make_drain1(b, q0c, LCc, aT_ps):
            # part a (jtx==0): copy the FULL accumulator out of PSUM (DVE).
            # This frees the PSUM bank immediately, so the next chunk's
            # PV (start=True at jtx==PIPE) never waits on the normalize.
            # part b (jtx==2): broadcast -> reciprocal -> normalize, all
            # reading the SBUF copy.
            st = {}

            def fa():
                # den row: equal-base copy; head blocks: PSUM base-0 in,
                # SBUF base h*64 out (up-shift, same as the proven
                # normalize pattern). NEVER shift a base DOWNWARD — custom
                # DVE ops silently read the wrong partitions.
                den = drpool.tile([65, HPC, LCc], BF16, tag="den",
                                  padded_shape=[65, HPC, 512])
                ac = drpool.tile([128, LCc], BF16, tag="ac",
                                 padded_shape=[128, 512])
                nc.vector.tensor_copy(den[64:65, :, :], aT_ps[64:65, :, :])
                for h in range(HPC):
                    nc.vector.tensor_copy(
                        ac[h * DH:(h + 1) * DH, :], aT_ps[0:DH, h, :]
                    )
                st["den"], st["ac"] = den, ac

            def fb():
                den, ac = st["den"], st["ac"]
                rep = psing.tile([128, LCc], F32, tag="single",
                                 padded_shape=[128, 512])
                for h in range(HPC):
                    nc.tensor.matmul(
                        rep[h * DH:(h + 1) * DH, :],
                        ones64[64:65, :],
                        den[64:65, h, :],
                        start=True, stop=True,
                        tile_position=(64, h * DH),
                    )
                rrec = drpool.tile([128, LCc], F32, tag="rrec",
                                   padded_shape=[128, 512])
                nc.vector.reciprocal_approx_fast(rrec[:, :], rep[:, :])
                for h in range(HPC):
                    nc.vector.tensor_mul(
                        aT_sb[b][h * DH:(h + 1) * DH, q0c:q0c + LCc],
                        ac[h * DH:(h + 1) * DH, :],
                        rrec[h * DH:(h + 1) * DH, :],
                    )
            return fa, fb

        def make_drain2(b, q0c, t, fine=False):
            # output projection for one 128-token tile of this query chunk.
            # fine=True (the very last tile): DMA each 512-col half as soon
            # as its copy lands, shortening the end-of-kernel flush.
            def f():
                tt = q0c // 128 + t
                ot = outpool.tile([128, D], BF16, tag="ot")
                for nch in range(2):
                    po = psing.tile([128, 512], F32, tag="single")
                    nc.tensor.matmul(
                        po[:, :],
                        aT_sb[b][:, tt * 128:(tt + 1) * 128],
                        wo_sb[:, nch * 512:(nch + 1) * 512],
                        start=True, stop=True,
                    )
                    nc.vector.tensor_copy(
                        ot[:, nch * 512:(nch + 1) * 512], po[:, :]
                    )
                    if fine:
                        nc.sync.dma_start(
                            out.ap()[b * Lb + tt * 128:b * Lb + (tt + 1) * 128,
                                     nch * 512:(nch + 1) * 512],
                            ot[:, nch * 512:(nch + 1) * 512],
                        )
                if not fine:
                    nc.sync.dma_start(
                        out.ap()[b * Lb + tt * 128:b * Lb + (tt + 1) * 128, :],
                        ot[:],
                    )
            return f

        # query chunks per batch: b1 tapers so the final (fully exposed)
        # drain chain covers only 128 queries instead of 512.
        def chunks_of(b):
            if Lb < 512:
                return [(i * 128, 128) for i in range(Lb // 128)]
            full = [(i * LC, LC) for i in range(NLC)]
            if b == B - 1:
                return full[:-1] + [(Lb - LC, 384), (Lb - 128, 128)]
            return full

        # software pipeline: scores/exp run PIPE jt-steps ahead of PV, so the
        # previous chunk's drain (spread over jtx 2..10) never starves ACT.
        PIPE = 4
        drain1a = drain1b = None
        drain2_pending = []
        deferred2 = deque()   # b0 outproj drains, parked for the dry phase
        deferred_hold = []    # not yet released: their drain1b hasn't run
        slot = 0
        for b in range(B):
            for (q0c, LCc) in chunks_of(b):
                q0 = b * Lb + q0c
                aT_ps = accpool.tile(
                    [65, HPC, LCc], F32, tag="acc", padded_shape=[65, HPC, 512]
                )
                ex_fifo = deque()
                for jtx in range(NJT + PIPE):
                    if jtx < NJT:
                        # consume projection work every slot, except when a
                        # drain piece already adds PE work to this slot
                        drain_here = (jtx == 2 and drain1b is not None) or \
                                     (jtx in (4, 6, 8, 10) and drain2_pending)
                        if not drain_here:
                            if queue:
                                pop_queue()
                                if slot < 2:
                                    pop_queue()  # front-load chunk-0 transposes
                            elif deferred2:
                                # queue dry: feed the PE parked b0 outproj
                                # work so late slots aren't ACT-bound
                                deferred2.popleft()()
                        slot += 1
                        k0 = b * Lb + jtx * 128
                        sc = scpool.tile(
                            [128, HPC, LCc], F32, tag="sc",
                            padded_shape=[128, HPC, 512],
                        )
                        for h in range(HPC):
                            nc.tensor.matmul(
                                sc[:, h, :],
                                kT_sb[h * DH:(h + 1) * DH, k0:k0 + 128],
                                qT_sb[h * DH:(h + 1) * DH, q0:q0 + LCc],
                                start=True, stop=True,
                                tile_position=(h * DH, 0),
                            )
                        ex = expool.tile([128, HPC, LCc], BF16, tag="ex",
                                         padded_shape=[128, HPC, 512])
                        nc.scalar.activation(ex[:], sc[:], AF.Exp)
                        ex_fifo.append(ex)
                    if jtx == 0 and drain1a is not None:
                        drain1a()
                        drain1a = None
                    if jtx == 2 and drain1b is not None:
                        drain1b()
                        drain1b = None
                        # normalize emitted: its outproj may now be parked
                        deferred2.extend(deferred_hold)
                        deferred_hold.clear()
                    if jtx in (4, 6, 8, 10) and drain2_pending:
                        drain2_pending.pop(0)()
                    if jtx >= PIPE:
                        jt = jtx - PIPE
                        tt = b * NJT + jt             # global 128-token tile
                        ex = ex_fifo.popleft()
                        for h in range(HPC):
                            nc.tensor.matmul(
                                aT_ps[:, h, :],
                                v_sb[:, tt, h, :],
                                ex[:, h, :],
                                start=(jt == 0), stop=(jt == NJT - 1),
                            )
                drain1a, drain1b = make_drain1(b, q0c, LCc, aT_ps)
                is_last = (b == B - 1) and (q0c + LCc == Lb)
                items = [
                    make_drain2(b, q0c, t, fine=is_last and t == LCc // 128 - 1)
                    for t in range(LCc // 128)
                ]
                if b == 0:
                    deferred_hold.extend(items)
                    drain2_pending = []
                else:
                    drain2_pending = items
        drain1a()
        drain1b()
        deferred2.extend(deferred_hold)
        while deferred2:
            deferred2.popleft()()
        for f in drain2_pending:
            f()
        while queue:
            pop_queue()

    nc.compile()
    return nc


_NC_CACHE = {}


def _get_nc(Lb=L):
    if Lb not in _NC_CACHE:
        _NC_CACHE[Lb] = build(Lb)
    return _NC_CACHE[Lb]


def make_in_maps(x, Wq, bq, Wk, bk, Wv, bv, Wo, bo, Lb=L):
    import ml_dtypes
    bf16 = ml_dtypes.bfloat16
    s = np.float32(DH ** (-0.25))
    BLb = B * Lb
    PC = min(512, BLb)
    NPC = BLb // PC
    # [128, NPC, KT, PC]: partition-major, chunk-contiguous
    xT = np.ascontiguousarray(
        np.asarray(x, np.float32).reshape(NPC, PC, KT, 128)
        .transpose(3, 0, 2, 1)
    ).astype(bf16)

    def wprep(w):   # [D, DHC] -> [128, KT, DHC]
        return np.ascontiguousarray(
            w.reshape(KT, 128, -1).transpose(1, 0, 2).astype(bf16)
        )

    Wq, Wk, Wv, Wo = (np.asarray(a, np.float32) for a in (Wq, Wk, Wv, Wo))
    bq, bk, bv = (np.asarray(a, np.float32) for a in (bq, bk, bv))
    in_maps = []
    for c in range(NCORES):
        hs = slice(c * DHC, (c + 1) * DHC)
        in_maps.append({
            "xT": xT,
            "wq": wprep(Wq[:, hs] * s),
            "wk": wprep(Wk[:, hs] * s),
            "wv": wprep(Wv[:, hs]),
            "wo": np.ascontiguousarray(Wo[hs, :].astype(bf16)),
            "bq": np.ascontiguousarray((bq[hs] * s).reshape(DHC, 1)),
            "bk": np.ascontiguousarray((bk[hs] * s).reshape(DHC, 1)),
            "bv": np.ascontiguousarray(bv[hs].reshape(DHC, 1)),
        })
    return in_maps


def kernel(x, Wq, bq, Wk, bk, Wv, bv, Wo, bo, **run_kwargs):
    x = np.asarray(x, np.float32)
    nc = _get_nc(L)
    in_maps = make_in_maps(x, Wq, bq, Wk, bk, Wv, bv, Wo, bo, L)
    res = bass_utils.run_bass_kernel_spmd(nc, in_maps, list(range(NCORES)), **run_kwargs)
    acc = np.zeros((B * L, D), np.float32)
    for r in res.results:
        acc += np.asarray(r["out"], np.float32)
    acc += np.asarray(bo, np.float32)[None, :]
    out = acc.reshape(B, L, D)
    kernel.last_results = res
    return out

